# revision 1
# baseline (speedup 1.0000x reference)
"""AttentionBlock (GroupNorm + single-head self-attention + residual) on 8 trn2 cores.

Data-parallel over batch: B=16 images, 2 per core; no collectives. All large
matmuls run as fp32r (FP22-truncated fp32), which streams at 1 col/cycle on
the PE for free dims >= 256 -- full bf16-rate with ~13-bit mantissa accuracy
(measured end-to-end rel err ~2.5e-5 vs the fp32 reference).

The four 512x512 projections are algebraically merged HOST-SIDE into two:
  logits  = scale * q^T k = hn^T (scale * wq^T wk) hn   -> one u-projection
  output  = wo @ (attn @ v) = attn @ ((wo wv) @ hn)     -> one v'-projection
so the device runs only 2 projection passes (u, v'T), no separate k/v
projections and NO output-projection phase at all. A nonzero bq adds a rank-1
per-column logit term; it is handled exactly (when present) as a per-m-tile
exp() bias computed by tiny N=1 matmuls of hn against scale * wk^T bq. bk is
dropped (constant per softmax row); bv folds into bo' = bo + wo@bv.

Per-image layouts (SBUF, partition x free):
  x, hn, u : [c, n] as 4 tiles [128, 1024]
  v'T      : [m, c'] as 8 tiles [128, 512] (hn-stationary matmuls, transposed
             for free)
  attnT    : exp(L^T)[m, n] as 8 tiles [128, 1024]

No PE transposes anywhere: logits are computed transposed (L^T = hn^T u per
128-row m-tile) so the ACT engine's Exp writes attnT straight from PSUM.
Softmax runs without max-subtraction (logits ~N(0,1) by construction).
Denominators: column sums of exp via ones-vector matmuls, broadcast to all
128 partitions with a K=1 outer-product matmul + fast approximate reciprocal;
the 1/sum multiply and the bias+residual add (x read from its GN-phase tiles) form the A@V'
PSUM->SBUF epilogue, streaming results out per [128, 512] chunk.

GroupNorm: per-channel sum/sumsq (DVE reduce + Square-accumulate split across
engines), group reduction and per-channel broadcast via tiny group-membership
matmuls. Both images' stats phases are emitted up front (tiny tiles double-
buffered); x loads are split across two DMA queues; a short burst of junk
fp32 matmuls warms the PE clock (HAM) during the initial DMA wait.
"""

import sys

sys.path.insert(0, "/opt/trn_rl_repo")

from contextlib import ExitStack

import numpy as np

import concourse.bass as bass
import concourse.bacc as bacc
import concourse.mybir as mybir
import concourse.tile as tile
from concourse.bass_utils import run_bass_kernel_spmd

B, C, H, W = 16, 512, 32, 32
HW = H * W  # 1024 pixels (n/m index)
NCORES = 8
BLOC = B // NCORES  # 2 images per core
G = 8  # groupnorm groups
GSZ = C // G  # 64 channels per group
SCALE = float(C) ** -0.5
EPS = 1e-5
INVCNT = 1.0 / (GSZ * HW)

F32 = mybir.dt.float32
F32R = mybir.dt.float32r
AF = mybir.ActivationFunctionType
ALU = mybir.AluOpType
AX = mybir.AxisListType

CT = C // 128  # 4 channel tiles
NB = HW // 128  # 8 row blocks of the attention matrix
NCH = HW // 512  # 2 free-dim chunks of 512


def r(ap):
    return ap.bitcast(F32R)


def _emit(tc, io):
    nc = tc.nc
    with ExitStack() as ctx, nc.allow_low_precision(reason="fp32r matmul operand rounding"):
        wp = ctx.enter_context(tc.tile_pool(name="wp", bufs=1))
        sb = ctx.enter_context(tc.tile_pool(name="sb", bufs=1))
        sp = ctx.enter_context(tc.tile_pool(name="sp", bufs=2))
        ps_l = ctx.enter_context(tc.tile_pool(name="ps_l", bufs=2, space="PSUM"))
        ps_m = ctx.enter_context(tc.tile_pool(name="ps_m", bufs=4, space="PSUM"))

        # ---- persistent weights / constants ----
        def load_w(key):
            ts = []
            for kt in range(CT):
                t = wp.tile([128, C], F32R, name=f"{key}{kt}", tag=f"{key}{kt}")
                nc.sync.dma_start(t[:], io[key][kt * 128 : (kt + 1) * 128, :])
                ts.append(t)
            return ts

        # PE warmup: the array sits idle ~13us waiting on x-DMA + GN stats and
        # would start cold (HAM 1.2GHz). Fill the window with junk fp32 matmuls
        # so the 3.4us activity window is warm before real work arrives.
        wsrc = wp.tile([128, 512], F32, name="wsrc", tag="wsrc")
        nc.vector.memset(wsrc[:], 0.0)
        warm_ps = ps_m.tile([128, 512], F32, name="warm_ps", tag="mm")
        for _ in range(4):
            nc.tensor.matmul(
                warm_ps[:], wsrc[:, 0:128], wsrc[:], start=True, stop=True
            )

        gmask_sb = []
        for kt in range(CT):
            t = wp.tile([128, G], F32R, name=f"gmask{kt}", tag=f"gmask{kt}")
            nc.sync.dma_start(t[:], io["gmask"][kt * 128 : (kt + 1) * 128, :])
            gmask_sb.append(t)
        gmaskT_sb = wp.tile([G, C], F32R, name="gmaskT", tag="gmaskT")
        nc.sync.dma_start(gmaskT_sb[:], io["gmaskT"][:])
        onescol = wp.tile([128, 1], F32R, name="onescol", tag="onescol")
        nc.sync.dma_start(onescol[:], io["onescol"][:])

        vecs_sb = wp.tile([128, CT * 4], F32, name="vecs", tag="vecs")
        nc.sync.dma_start(
            vecs_sb[:].rearrange("p (t f) -> p t f", t=CT),
            io["vecs"].rearrange("(t p) f -> p t f", p=128),
        )

        def vcol(ct, f):
            return vecs_sb[:, ct * 4 + f : ct * 4 + f + 1]

        ones1 = wp.tile([1, 128], F32R, name="ones1", tag="ones1")
        nc.sync.dma_start(ones1[:], io["ones1"][:])

        wu_sb = load_w("wuT")
        wvo_sb = load_w("wvoT")
        w2_sb = None
        if io.get("w2col") is not None:
            w2_sb = []
            for kt in range(CT):
                t = wp.tile([128, 1], F32R, name=f"w2c{kt}", tag=f"w2c{kt}")
                nc.sync.dma_start(t[:], io["w2col"][kt * 128 : (kt + 1) * 128, :])
                w2_sb.append(t)

        def stats_phase(img):
            # ---- load x ----
                xt = []
                for ct in range(CT):
                    t = sb.tile([128, HW], F32, name=f"xt{ct}", tag=f"xt{ct}", bufs=2)
                    nc.gpsimd.dma_start(t[:], io["x"][img, ct * 128 : (ct + 1) * 128, :])
                    xt.append(t)

                # ---- groupnorm stats: per-channel sum (DVE) and sumsq (ACT) ----
                stat2 = []
                for ct in range(CT):
                    s2 = sb.tile([128, 2], F32R, name=f"stat2_{ct}", tag=f"stat2_{ct}", bufs=2)
                    nc.vector.reduce_sum(s2[:, 0:1], xt[ct][:], axis=AX.X)
                    scr = sp.tile(
                        [128, HW], F32, name="scr", tag=f"scr{ct % 2}", bufs=1
                    )
                    if ct % 2 == 0 and img == 0:
                        nc.scalar.activation(
                            scr[:], xt[ct][:], AF.Square, accum_out=s2[:, 1:2]
                        )
                    else:
                        nc.vector.scalar_tensor_tensor(
                            scr[:], xt[ct][:], 1.0, xt[ct][:],
                            op0=ALU.mult, op1=ALU.mult,
                            accum_out=s2[:, 1:2],
                        )
                    stat2.append(s2)

                # group sums via membership-mask matmul: [8, 2]
                gstat = ps_m.tile([G, 2], F32, name="gstat", tag="mm")
                for ct in range(CT):
                    nc.tensor.matmul(
                        gstat[:],
                        r(gmask_sb[ct][:]),
                        r(stat2[ct][:]),
                        start=(ct == 0),
                        stop=(ct == CT - 1),
                    )
                gs = sb.tile([G, 2], F32, name="gs", tag="gs", bufs=2)
                nc.vector.tensor_copy(gs[:], gstat[:])

                # per-group mean / rstd, packed as grp2 = [mean, rstd]
                grp2 = sb.tile([G, 2], F32R, name="grp2", tag="grp2", bufs=2)
                tmx = sb.tile([G, 4], F32, name="tmx", tag="tmx", bufs=2)
                nc.vector.tensor_scalar_mul(grp2[:, 0:1], gs[:, 0:1], INVCNT)  # mean
                nc.vector.tensor_scalar_mul(tmx[:, 0:1], gs[:, 1:2], INVCNT)  # E[x^2]
                nc.vector.tensor_mul(tmx[:, 1:2], grp2[:, 0:1], grp2[:, 0:1])  # mean^2
                nc.vector.scalar_tensor_tensor(
                    tmx[:, 2:3], tmx[:, 0:1], EPS, tmx[:, 1:2],
                    op0=ALU.add, op1=ALU.subtract,
                )  # var + eps
                nc.vector.reciprocal(tmx[:, 3:4], tmx[:, 2:3])
                nc.scalar.sqrt(grp2[:, 1:2], tmx[:, 3:4])  # rstd

                # broadcast mean/rstd to channels, fold gamma/beta
                ac, bc = [], []
                for ct in range(CT):
                    bcp = ps_m.tile([128, 2], F32, name="bcp", tag="mm")
                    nc.tensor.matmul(
                        bcp[:],
                        r(gmaskT_sb[:, ct * 128 : (ct + 1) * 128]),
                        r(grp2[:]),
                        start=True,
                        stop=True,
                    )
                    a1 = sb.tile([128, 4], F32, name=f"ab{ct}", tag=f"ab{ct}", bufs=2)
                    # a = rstd * gamma ; b = beta - mean * a
                    nc.vector.tensor_mul(a1[:, 0:1], bcp[:, 1:2], vcol(ct, 1))
                    nc.vector.tensor_mul(a1[:, 2:3], bcp[:, 0:1], a1[:, 0:1])
                    nc.vector.tensor_sub(a1[:, 1:2], vcol(ct, 2), a1[:, 2:3])
                    ac.append(a1[:, 0:1])
                    bc.append(a1[:, 1:2])
                return xt, ac, bc

        per_img = [stats_phase(img) for img in range(BLOC)]

        for img in range(BLOC):
            xt, ac, bc = per_img[img]
            # hn = x * a + b   (DVE two-op tensor_scalar)
            hn = []
            for ct in range(CT):
                t = sb.tile([128, HW], F32R, name=f"hn{ct}", tag=f"hn{ct}")
                if ct % 2 == 0:
                    nc.vector.tensor_scalar(
                        t[:], xt[ct][:], ac[ct], bc[ct], op0=ALU.mult, op1=ALU.add
                    )
                else:
                    nc.scalar.activation(
                        t[:], xt[ct][:], AF.Identity, bias=bc[ct], scale=ac[ct]
                    )
                hn.append(t)

            # ---- u projection: u = (scale * wk^T wq) @ hn, so L = u^T hn ----
            u_sb = []
            for cc in range(CT):
                dst = sb.tile([128, HW], F32R, name=f"u{cc}", tag=f"u{cc}")
                accs = [
                    ps_m.tile([128, 512], F32, name="qp", tag="mm")
                    for _ in range(NCH)
                ]
                for kt in range(CT):
                    for nch in range(NCH):
                        nc.tensor.matmul(
                            accs[nch][:],
                            r(wu_sb[kt][:, cc * 128 : (cc + 1) * 128]),
                            r(hn[kt][:, nch * 512 : (nch + 1) * 512]),
                            start=(kt == 0),
                            stop=(kt == CT - 1),
                        )
                for nch in range(NCH):
                    dslice = dst[:, nch * 512 : (nch + 1) * 512]
                    if (cc + nch) % 2 == 0:
                        nc.vector.tensor_copy(dslice, accs[nch][:])
                    else:
                        nc.scalar.copy(dslice, accs[nch][:])
                u_sb.append(dst)

            # ---- v'T: [m, c'] with v' = (wo @ wv) @ hn (projection pre-merged) ----
            vT = [None] * NB

            def emit_vT(mts):
                for mt in mts:
                    dst = sb.tile([128, C], F32R, name=f"vT{mt}", tag=f"vT{mt}")
                    acc = ps_m.tile([128, 512], F32, name="vp", tag="mm")
                    for kt in range(CT):
                        nc.tensor.matmul(
                            acc[:],
                            r(hn[kt][:, mt * 128 : (mt + 1) * 128]),
                            r(wvo_sb[kt][:]),
                            start=(kt == 0),
                            stop=(kt == CT - 1),
                        )
                    if mt % 2 == 0:
                        nc.vector.tensor_copy(dst[:], acc[:])
                    else:
                        nc.scalar.copy(dst[:], acc[:])
                    vT[mt] = dst


            # optional per-m logit offset for nonzero bq: c_m = (scale wk^T bq) . hn[:, m]
            tv_sb = None
            if w2_sb is not None:
                tv_sb = []
                for mt in range(NB):
                    tvp = ps_m.tile([128, 1], F32, name="tvp", tag="mm")
                    for kt in range(CT):
                        nc.tensor.matmul(
                            tvp[:],
                            r(hn[kt][:, mt * 128 : (mt + 1) * 128]),
                            r(w2_sb[kt][:]),
                            start=(kt == 0),
                            stop=(kt == CT - 1),
                        )
                    t = sb.tile([128, 1], F32, name=f"tv{mt}", tag=f"tv{mt}", bufs=2)
                    nc.vector.tensor_copy(t[:], tvp[:])
                    tv_sb.append(t)

            # ---- attention: L^T = hn^T u per m-tile; exp writes attnT from PSUM ----
            attnT = []
            for mt in range(NB):
                t = sb.tile([128, HW], F32R, name=f"attnT{mt}", tag=f"attnT{mt}")
                attnT.append(t)
            for mt in range(NB):
                lpT = ps_l.tile([128, HW], F32, name="lpT", tag="lpT")
                for kt in range(CT):
                    for nch in range(NCH):
                        nc.tensor.matmul(
                            lpT[:, nch * 512 : (nch + 1) * 512],
                            r(hn[kt][:, mt * 128 : (mt + 1) * 128]),
                            r(u_sb[kt][:, nch * 512 : (nch + 1) * 512]),
                            start=(kt == 0),
                            stop=(kt == CT - 1),
                        )
                if tv_sb is not None:
                    nc.scalar.activation(
                        attnT[mt][:], lpT[:], AF.Exp, bias=tv_sb[mt][:]
                    )
                else:
                    nc.scalar.activation(attnT[mt][:], lpT[:], AF.Exp)

            emit_vT(range(NB))
            # softmax denominators: column sums via ones-vector matmuls, then
            # 1/sum broadcast rows rb[h] via outer product + fast reciprocal
            cs_t = []
            for half in range(2):
                hsl = slice(half * 512, (half + 1) * 512)
                cs = ps_m.tile([1, 512], F32, name="cs", tag="mm")
                for mt in range(NB):
                    nc.tensor.matmul(
                        cs[:],
                        r(onescol[:]),
                        r(attnT[mt][:, hsl]),
                        start=(mt == 0),
                        stop=(mt == NB - 1),
                    )
                cs_t.append(cs)
            rb_sb = []
            for half in range(2):
                rrow_sb = sp.tile(
                    [1, 512], F32R, name="rrow_sb", tag="rrow_sb", bufs=2
                )
                nc.vector.tensor_copy(rrow_sb[:], cs_t[half][:])
                rb_ps = ps_m.tile([128, 512], F32, name="rb_ps", tag="mm")
                nc.tensor.matmul(
                    rb_ps[:], r(ones1[:]), r(rrow_sb[:]), start=True, stop=True
                )
                t = sp.tile([128, 512], F32, name=f"rb{half}", tag=f"rb{half}", bufs=1)
                nc.vector.reciprocal_approx_fast(t[:], rb_ps[:])
                rb_sb.append(t)

            # ---- A @ V': directly the projected attention output; epilogue
            # normalizes, adds bias + residual, and streams out ----
            for cc in range(CT):
                accs = [
                    ps_m.tile([128, 512], F32, name="op", tag="mm")
                    for _ in range(2)
                ]
                for mt in range(NB):
                    for half in range(2):
                        nc.tensor.matmul(
                            accs[half][:],
                            r(vT[mt][:, cc * 128 : (cc + 1) * 128]),
                            r(attnT[mt][:, half * 512 : (half + 1) * 512]),
                            start=(mt == 0),
                            stop=(mt == NB - 1),
                        )
                for half in range(2):
                    hsl = slice(half * 512, (half + 1) * 512)
                    on = sp.tile([128, 512], F32, name="on", tag="on", bufs=3)
                    nc.vector.tensor_mul(on[:], accs[half][:], rb_sb[half][:])
                    res = sp.tile([128, 512], F32, name="res", tag="res", bufs=3)
                    nc.vector.scalar_tensor_tensor(
                        res[:],
                        on[:],
                        vcol(cc, 3),
                        xt[cc][:, hsl],
                        op0=ALU.add,
                        op1=ALU.add,
                    )
                    out_eng = nc.sync if (cc + half) % 2 == 0 else nc.gpsimd
                    out_eng.dma_start(
                        io["out"][img, cc * 128 : (cc + 1) * 128, hsl],
                        res[:],
                    )


_NC = {}


def _build(has_bq=False):
    global _NC
    if _NC.get(has_bq) is None:
        nc = bacc.Bacc("TRN2", target_bir_lowering=False, debug=False)
        io = {}
        io["x"] = nc.dram_tensor("x", [BLOC, C, HW], F32, kind="ExternalInput").ap()
        for key in ("wuT", "wvoT"):
            io[key] = nc.dram_tensor(key, [C, C], F32R, kind="ExternalInput").ap()
        if has_bq:
            io["w2col"] = nc.dram_tensor(
                "w2col", [C, 1], F32R, kind="ExternalInput"
            ).ap()
        io["gmask"] = nc.dram_tensor("gmask", [C, G], F32R, kind="ExternalInput").ap()
        io["gmaskT"] = nc.dram_tensor("gmaskT", [G, C], F32R, kind="ExternalInput").ap()
        io["onescol"] = nc.dram_tensor("onescol", [128, 1], F32R, kind="ExternalInput").ap()
        io["ones1"] = nc.dram_tensor("ones1", [1, 128], F32R, kind="ExternalInput").ap()
        io["vecs"] = nc.dram_tensor("vecs", [C, 4], F32, kind="ExternalInput").ap()
        io["out"] = nc.dram_tensor("out", [BLOC, C, HW], F32, kind="ExternalOutput").ap()
        with tile.TileContext(nc, pool_alloc_mode="queue") as tc:
            _emit(tc, io)
        nc.compile()
        _NC[has_bq] = nc
    return _NC[has_bq]


def _host_prep(x, gn_w, gn_b, wq, bq, wk, bk, wv, bv, wo, bo):
    f = np.float32
    wq64 = np.asarray(wq, np.float64)
    wk64 = np.asarray(wk, np.float64)
    wv64 = np.asarray(wv, np.float64)
    wo64 = np.asarray(wo, np.float64)
    has_bq = bool(np.any(np.asarray(bq) != 0))
    shared = {
        "wuT": np.ascontiguousarray(SCALE * (wq64.T @ wk64), dtype=f),
        "wvoT": np.ascontiguousarray((wo64 @ wv64).T, dtype=f),
        "vecs": np.ascontiguousarray(
            np.stack(
                [
                    np.asarray(bq, dtype=f),
                    np.asarray(gn_w, dtype=f),
                    np.asarray(gn_b, dtype=f),
                    (bo + wo @ bv).astype(f),
                ],
                axis=1,
            )
        ),
        "gmask": np.repeat(np.eye(G, dtype=f), GSZ, axis=0),
        "gmaskT": np.ascontiguousarray(np.repeat(np.eye(G, dtype=f), GSZ, axis=0).T),
        "onescol": np.ones((128, 1), dtype=f),
        "ones1": np.ones((1, 128), dtype=f),
    }
    if has_bq:
        shared["w2col"] = np.ascontiguousarray(
            (SCALE * (wk64.T @ np.asarray(bq, np.float64)))[:, None], dtype=f
        )
    xr = np.ascontiguousarray(np.asarray(x, dtype=f).reshape(B, C, HW))
    in_maps = []
    for core in range(NCORES):
        m = dict(shared)
        m["x"] = np.ascontiguousarray(xr[core * BLOC : (core + 1) * BLOC])
        in_maps.append(m)
    return in_maps


def _run(inputs, trace=False, **kw):
    in_maps = _host_prep(**inputs)
    nc = _build(has_bq="w2col" in in_maps[0])
    res = run_bass_kernel_spmd(
        nc, in_maps, core_ids=list(range(NCORES)), trace=trace, **kw
    )
    outs = [np.asarray(res.results[i]["out"]) for i in range(NCORES)]
    full = np.concatenate(outs, axis=0).reshape(B, C, H, W).astype(np.float32)
    return full, res


def kernel(**inputs):
    full, _ = _run(inputs, trace=False)
    return full



# revision 10
# speedup vs baseline: 1.2859x; 1.2859x over previous
"""AttentionBlock (GroupNorm + single-head self-attention + residual) on 8 trn2 cores.

Data-parallel over batch: B=16 images, 2 per core; no collectives. All large
matmuls run as fp8-e4m3 with perf_mode=DoubleRow: operands are packed
[128, planes, free] and each matmul contracts 2 planes (K=256) at once,
~1.4-2x the fp32r/bf16 PE rate. Host-side scaling keeps every fp8 operand at
unit-ish sigma (wuT x512, wvoT x16), compensated exactly on-device by the exp
scale (1/512) and by folding 1/16 into the softmax-denominator reciprocal.
Measured end-to-end rel err ~5e-3 vs the fp32 reference (tolerance 2e-2).

The four 512x512 projections are algebraically merged HOST-SIDE into two:
  logits  = scale * q^T k = hn^T (scale * wq^T wk) hn   -> one u-projection
  output  = wo @ (attn @ v) = attn @ ((wo wv) @ hn)     -> one v'-projection
bk is dropped (constant per softmax row); bv folds into bo' = bo + wo@bv; a
nonzero bq becomes a per-m exp() bias from tiny matmuls of hn against
scale * wk^T bq (exact; bq==0 in practice skips it).

Per-image fp8 layouts (SBUF, partition x planes x free):
  hn, u  : [128, 4, 1024]  (channel planes x pixels)
  v'T    : [128, 8, 512]   (pixel planes x channels)
  attnT  : [128, 8, 1024]  (exp(L^T), pixel planes x pixels)
Softmax runs without max-subtraction (logits ~N(0,1) by construction); exp
uses bias -1.5 to keep fp8 attnT well inside e4m3 range. Denominators are
column sums of the *stored* fp8 attnT via ones-vector DoubleRow matmuls
(exact normalization consistency), broadcast to 128 partitions by a K=1
outer-product matmul + fast reciprocal.

GroupNorm runs in fp32: bn_stats/bn_aggr (one DVE pass per x tile) give
per-channel mean/var; group reduction and per-channel broadcast go through
tiny group-membership matmuls. x loads split across the two HWDGE queues
(sync/scalar), weights ride SWDGE (gpsimd/vector); a burst of junk matmuls
warms the PE clock (HAM) during the initial DMA wait. Image 1's hn is
produced mid-way through image 0's attention so the PE never idles between
images; the epilogue (1/denominator, +bias, +residual) streams per
[128, 512] chunk on DVE/GPSIMD straight into output DMAs on 4 queues.
"""

import sys

sys.path.insert(0, "/opt/trn_rl_repo")

from contextlib import ExitStack

import numpy as np
import ml_dtypes

import concourse.bass as bass
import concourse.bacc as bacc
import concourse.mybir as mybir
import concourse.tile as tile
from concourse.bass_utils import run_bass_kernel_spmd

B, C, H, W = 16, 512, 32, 32
HW = H * W  # 1024 pixels (n/m index)
NCORES = 8
BLOC = B // NCORES  # 2 images per core
G = 8  # groupnorm groups
GSZ = C // G  # 64 channels per group
SCALE = float(C) ** -0.5
EPS = 1e-5
INVG = 1.0 / GSZ

SU = 512.0  # host scale on wuT; exp() divides it back out
SV = 16.0  # host scale on wvoT; folded into denominator reciprocal
SU2 = 64.0  # host scale on the bq logit-offset column
EXPB = -1.5  # exp bias: keeps fp8 attnT in e4m3 range (cancels in softmax)
F8MAX = 240.0  # TRN e4m3 max normal

F32 = mybir.dt.float32
F32R = mybir.dt.float32r
F8 = mybir.dt.float8e4
NF8 = ml_dtypes.float8_e4m3
AF = mybir.ActivationFunctionType
ALU = mybir.AluOpType
AX = mybir.AxisListType
DR = mybir.MatmulPerfMode.DoubleRow

CT = C // 128  # 4 channel tiles
CP = CT // 2  # 2 channel-tile pairs (DoubleRow K=256)
NB = HW // 128  # 8 row blocks of the attention matrix
NP = NB // 2  # 4 row-block pairs
NCH = HW // 512  # 2 free-dim chunks of 512

NWARM = 10  # junk PE warmup matmuls covering the x-DMA + stats window


def r(ap):
    return ap.bitcast(F32R)


def _emit(tc, io):
    nc = tc.nc
    with ExitStack() as ctx, nc.allow_low_precision(reason="fp8 attention"):
        wp = ctx.enter_context(tc.tile_pool(name="wp", bufs=1))
        sb = ctx.enter_context(tc.tile_pool(name="sb", bufs=1))
        sp = ctx.enter_context(tc.tile_pool(name="sp", bufs=2))
        ps_l = ctx.enter_context(tc.tile_pool(name="ps_l", bufs=2, space="PSUM"))
        ps_m = ctx.enter_context(tc.tile_pool(name="ps_m", bufs=4, space="PSUM"))

        # ---- x loads first, split across both HWDGE queues (hot path) ----
        xt = [[None] * CT for _ in range(BLOC)]
        for img in range(BLOC):
            for ct in range(CT):
                t = sb.tile([128, HW], F32, name=f"x{img}_{ct}", tag=f"x{img}_{ct}")
                xt[img][ct] = t
                q = nc.sync if ct % 2 == 0 else nc.scalar
                q.dma_start(t[:], io["x"][img, ct * 128 : (ct + 1) * 128, :])

        # ---- weights / constants on the SWDGE queues ----
        wu8 = wp.tile([128, CT, C], F8, name="wu8", tag="wu8")
        nc.gpsimd.dma_start(wu8[:], io["wu8"][:])
        wvo8 = wp.tile([128, CT, C], F8, name="wvo8", tag="wvo8")
        nc.gpsimd.dma_start(wvo8[:], io["wvo8"][:])
        gmask_sb = []
        for kt in range(CT):
            t = wp.tile([128, G], F32R, name=f"gmask{kt}", tag=f"gmask{kt}")
            nc.gpsimd.dma_start(t[:], io["gmask"][kt * 128 : (kt + 1) * 128, :])
            gmask_sb.append(t)
        gmaskT_sb = wp.tile([G, C], F32R, name="gmaskT", tag="gmaskT")
        nc.gpsimd.dma_start(gmaskT_sb[:], io["gmaskT"][:])
        ones8 = wp.tile([128, 2, 16], F8, name="ones8", tag="ones8")
        nc.gpsimd.dma_start(ones8[:], io["ones8"][:])
        ones1 = wp.tile([1, 128], F32R, name="ones1", tag="ones1")
        nc.gpsimd.dma_start(ones1[:], io["ones1"][:])
        vecs_sb = wp.tile([128, CT * 4], F32, name="vecs", tag="vecs")
        nc.gpsimd.dma_start(
            vecs_sb[:].rearrange("p (t f) -> p t f", t=CT),
            io["vecs"].rearrange("(t p) f -> p t f", p=128),
        )
        w2_sb = None
        if io.get("w28") is not None:
            w2_sb = wp.tile([128, CT, 16], F8, name="w28", tag="w28")
            nc.gpsimd.dma_start(w2_sb[:], io["w28"][:])

        def vcol(ct, f):
            return vecs_sb[:, ct * 4 + f : ct * 4 + f + 1]

        # PE warmup: the array sits idle ~5us waiting on x-DMA + GN stats and
        # would start cold (HAM 1.2GHz). Fill the window with junk fp32r
        # matmuls so the 3.4us activity window is warm when real work arrives.
        wsrc = wp.tile([128, 512], F32, name="wsrc", tag="wsrc")
        nc.vector.memset(wsrc[:], 0.0)
        expb = wp.tile([128, 1], F32, name="expb", tag="expb")
        nc.vector.memset(expb[:], EXPB)
        warm_ps = ps_m.tile([128, 512], F32, name="warm_ps", tag="mm")
        for _ in range(NWARM):
            nc.tensor.matmul(
                warm_ps[:], r(wsrc[:, 0:128]), r(wsrc[:]), start=True, stop=True
            )

        # ---- groupnorm stats: bn_stats/bn_aggr per x tile, group matmuls ----
        def stats_phase(img):
            sts = []
            for ct in range(CT):
                bn6 = sp.tile([128, 12], F32, name="bn6", tag=f"bn6_{ct}", bufs=2)
                for b in range(2):
                    nc.vector.bn_stats(
                        bn6[:, b * 6 : (b + 1) * 6],
                        xt[img][ct][:, b * 512 : (b + 1) * 512],
                    )
                mv = sp.tile([128, 2], F32, name="mv", tag=f"mv_{ct}", bufs=2)
                nc.vector.bn_aggr(mv[:], bn6[:])
                # st = [mean, E[x^2]] per channel
                st = sp.tile([128, 2], F32R, name="st", tag=f"st_{ct}", bufs=2)
                nc.vector.tensor_copy(st[:, 0:1], mv[:, 0:1])
                nc.vector.scalar_tensor_tensor(
                    st[:, 1:2], mv[:, 0:1], mv[:, 0:1], mv[:, 1:2],
                    op0=ALU.mult, op1=ALU.add,
                )
                sts.append(st)

            # group sums via membership-mask matmul: [8, 2]
            gstat = ps_m.tile([G, 2], F32, name="gstat", tag="mm")
            for ct in range(CT):
                nc.tensor.matmul(
                    gstat[:], gmask_sb[ct][:], r(sts[ct][:]),
                    start=(ct == 0), stop=(ct == CT - 1),
                )
            gs = sp.tile([G, 2], F32, name="gs", tag="gs", bufs=2)
            nc.vector.tensor_copy(gs[:], gstat[:])

            # per-group mean / rstd, packed as grp2 = [mean, rstd]
            grp2 = sp.tile([G, 2], F32R, name="grp2", tag="grp2", bufs=2)
            tmx = sp.tile([G, 4], F32, name="tmx", tag="tmx", bufs=2)
            nc.vector.tensor_scalar_mul(grp2[:, 0:1], gs[:, 0:1], INVG)  # mean
            nc.vector.tensor_scalar_mul(tmx[:, 0:1], gs[:, 1:2], INVG)  # E[x^2]
            nc.vector.tensor_mul(tmx[:, 1:2], grp2[:, 0:1], grp2[:, 0:1])  # mean^2
            nc.vector.scalar_tensor_tensor(
                tmx[:, 2:3], tmx[:, 0:1], EPS, tmx[:, 1:2],
                op0=ALU.add, op1=ALU.subtract,
            )  # var + eps
            nc.vector.reciprocal(tmx[:, 3:4], tmx[:, 2:3])
            nc.scalar.sqrt(grp2[:, 1:2], tmx[:, 3:4])  # rstd

            # broadcast mean/rstd to channels, fold gamma/beta
            ac, bc = [], []
            for ct in range(CT):
                bcp = ps_m.tile([128, 2], F32, name="bcp", tag="mm")
                nc.tensor.matmul(
                    bcp[:], gmaskT_sb[:, ct * 128 : (ct + 1) * 128], r(grp2[:]),
                    start=True, stop=True,
                )
                a1 = sp.tile([128, 4], F32, name=f"ab{img}{ct}", tag=f"ab{img}{ct}", bufs=1)
                # a = rstd * gamma ; b = beta - mean * a
                nc.vector.tensor_mul(a1[:, 0:1], bcp[:, 1:2], vcol(ct, 1))
                nc.vector.tensor_mul(a1[:, 2:3], bcp[:, 0:1], a1[:, 0:1])
                nc.vector.tensor_sub(a1[:, 1:2], vcol(ct, 2), a1[:, 2:3])
                ac.append(a1[:, 0:1])
                bc.append(a1[:, 1:2])
            return ac, bc

        per_img = [stats_phase(img) for img in range(BLOC)]

        # hn = x * a + b, written fp8 into plane ct of the packed tile
        hns = [None] * BLOC

        def emit_hn(img, engines):
            ac, bc = per_img[img]
            hnp = sb.tile([128, CT, HW], F8, name=f"hn{img}", tag=f"hn{img}")
            for ct in range(CT):
                dst = hnp[:, ct, :]
                eng = engines[ct % len(engines)]
                if eng is nc.scalar:
                    nc.scalar.activation(
                        dst, xt[img][ct][:], AF.Identity, bias=bc[ct], scale=ac[ct]
                    )
                else:
                    eng.tensor_scalar(
                        dst, xt[img][ct][:], ac[ct], bc[ct], op0=ALU.mult, op1=ALU.add
                    )
            hns[img] = hnp

        def pcopy(i, dst, src):
            # PSUM -> SBUF copy (with fp8 convert); GPSIMD can't read PSUM
            if i % 2 == 0:
                nc.vector.tensor_copy(dst, src)
            else:
                nc.scalar.copy(dst, src)

        def compute_phase(img, mid_hook=None):
            hnp = hns[img]

            # optional per-m logit offset for nonzero bq
            tv_sb = None
            if w2_sb is not None:
                tv_sb = []
                for mt in range(NB):
                    tvp = ps_m.tile([128, 1], F32, name="tvp", tag="mm")
                    for t in range(CP):
                        nc.tensor.matmul(
                            tvp[:],
                            hnp[:, 2 * t : 2 * t + 2, mt * 128 : (mt + 1) * 128],
                            w2_sb[:, 2 * t : 2 * t + 2, 0:1],
                            start=(t == 0), stop=(t == CP - 1), perf_mode=DR,
                        )
                    tb = sp.tile([128, 1], F32, name=f"tv{mt}", tag=f"tv{mt}", bufs=2)
                    nc.vector.tensor_scalar(
                        tb[:], tvp[:], 1.0 / SU2, EXPB, op0=ALU.mult, op1=ALU.add
                    )
                    tv_sb.append(tb)

            # ---- u projection: u = (SU * scale * wk^T wq) @ hn ----
            up = sb.tile([128, CT, HW], F8, name=f"u{img}", tag=f"u{img}")
            ci = 0
            for cc in range(CT):
                accs = [
                    ps_m.tile([128, 512], F32, name="qp", tag="mm") for _ in range(NCH)
                ]
                for t in range(CP):
                    for nch in range(NCH):
                        nc.tensor.matmul(
                            accs[nch][:],
                            wu8[:, 2 * t : 2 * t + 2, cc * 128 : (cc + 1) * 128],
                            hnp[:, 2 * t : 2 * t + 2, nch * 512 : (nch + 1) * 512],
                            start=(t == 0), stop=(t == CP - 1), perf_mode=DR,
                        )
                for nch in range(NCH):
                    pcopy(ci, up[:, cc, nch * 512 : (nch + 1) * 512], accs[nch][:])
                    ci += 1

            # xb = x + bo' (residual + output bias), consumed by the epilogue;
            # GPSIMD lacks scalar-ptr ops so this folds the per-channel bias
            # here on ACT/DVE instead
            xb = []
            for ct in range(CT):
                t = sb.tile([128, HW], F32, name=f"xb{img}_{ct}", tag=f"xb{img}_{ct}")
                if ct % 2 == 0:
                    nc.scalar.activation(
                        t[:], xt[img][ct][:], AF.Identity, bias=vcol(ct, 3)
                    )
                else:
                    nc.vector.tensor_scalar_add(t[:], xt[img][ct][:], vcol(ct, 3))
                xb.append(t)

            # ---- v'T: [m, c'] with v' = (SV * wo wv) @ hn ----
            vTp = sb.tile([128, NB, C], F8, name=f"vT{img}", tag=f"vT{img}")

            def emit_vT(mts):
                nonlocal ci
                for mt in mts:
                    acc = ps_m.tile([128, 512], F32, name="vp", tag="mm")
                    for t in range(CP):
                        nc.tensor.matmul(
                            acc[:],
                            hnp[:, 2 * t : 2 * t + 2, mt * 128 : (mt + 1) * 128],
                            wvo8[:, 2 * t : 2 * t + 2, :],
                            start=(t == 0), stop=(t == CP - 1), perf_mode=DR,
                        )
                    pcopy(ci, vTp[:, mt, :], acc[:])
                    ci += 1

            emit_vT(range(0, NB // 2))

            # ---- attention: L^T = hn^T u per m-tile; exp writes fp8 attnT ----
            attnp = sb.tile([128, NB, HW], F8, name=f"at{img}", tag=f"at{img}")
            for mt in range(NB):
                lpT = ps_l.tile([128, HW], F32, name="lpT", tag="lpT")
                for t in range(CP):
                    for nch in range(NCH):
                        nc.tensor.matmul(
                            lpT[:, nch * 512 : (nch + 1) * 512],
                            hnp[:, 2 * t : 2 * t + 2, mt * 128 : (mt + 1) * 128],
                            up[:, 2 * t : 2 * t + 2, nch * 512 : (nch + 1) * 512],
                            start=(t == 0), stop=(t == CP - 1), perf_mode=DR,
                        )
                if tv_sb is not None:
                    nc.scalar.activation(
                        attnp[:, mt, :], lpT[:], AF.Exp, bias=tv_sb[mt][:], scale=1.0 / SU
                    )
                else:
                    nc.scalar.activation(
                        attnp[:, mt, :], lpT[:], AF.Exp, bias=expb[:], scale=1.0 / SU
                    )

            if mid_hook is not None:
                mid_hook()

            emit_vT(range(NB // 2, NB))

            # softmax denominators: column sums of stored fp8 attnT, then
            # 1/(SV*sum) broadcast to all partitions via outer-product matmul
            cs_t = []
            for half in range(2):
                hsl = slice(half * 512, (half + 1) * 512)
                cs = ps_m.tile([1, 512], F32, name="cs", tag="mm")
                for j in range(NP):
                    nc.tensor.matmul(
                        cs[:],
                        ones8[:, :, 0:1],
                        attnp[:, 2 * j : 2 * j + 2, hsl],
                        start=(j == 0), stop=(j == NP - 1), perf_mode=DR,
                    )
                cs_t.append(cs)
            rrows = []
            for half in range(2):
                rrow = sp.tile([1, 512], F32R, name="rrow", tag=f"rrow{half}", bufs=2)
                nc.vector.tensor_copy(rrow[:], cs_t[half][:])
                rrows.append(rrow)
            rb_sb = []
            for half in range(2):
                rb_ps = ps_m.tile([128, 512], F32, name="rb_ps", tag="mm")
                nc.tensor.matmul(
                    rb_ps[:], ones1[:], rrows[half][:], start=True, stop=True
                )
                t = sp.tile([128, 512], F32, name=f"rb{half}", tag=f"rb{half}", bufs=1)
                nc.vector.reciprocal_approx_fast(t[:], rb_ps[:])
                rb_sb.append(t)

            # ---- A @ V': epilogue normalizes, adds bias + residual, streams out
            outqs = (nc.sync, nc.scalar, nc.gpsimd, nc.sync)
            for cc in range(CT):
                accs = [
                    ps_m.tile([128, 512], F32, name="op", tag="mm") for _ in range(2)
                ]
                for j in range(NP):
                    for half in range(2):
                        nc.tensor.matmul(
                            accs[half][:],
                            vTp[:, 2 * j : 2 * j + 2, cc * 128 : (cc + 1) * 128],
                            attnp[:, 2 * j : 2 * j + 2, half * 512 : (half + 1) * 512],
                            start=(j == 0), stop=(j == NP - 1), perf_mode=DR,
                        )
                for half in range(2):
                    hsl = slice(half * 512, (half + 1) * 512)
                    on = sp.tile([128, 512], F32, name="on", tag="on", bufs=3)
                    nc.vector.tensor_mul(on[:], accs[half][:], rb_sb[half][:])
                    res = sp.tile([128, 512], F32, name="res", tag="res", bufs=3)
                    add_eng = nc.gpsimd if (cc + half) % 2 == 0 else nc.vector
                    add_eng.tensor_add(res[:], on[:], xb[cc][:, hsl])
                    outqs[(cc * 2 + half) % 4].dma_start(
                        io["out"][img, cc * 128 : (cc + 1) * 128, hsl], res[:]
                    )

        emit_hn(0, (nc.vector, nc.scalar))
        # image 1's hn is produced mid-way through image 0's attention phase
        # (on otherwise-idle engines) so u1 matmuls can start the moment the
        # PE finishes image 0's A@V.
        compute_phase(0, mid_hook=lambda: emit_hn(1, (nc.vector,)))
        compute_phase(1)


_NC = {}


def _build(has_bq=False):
    global _NC
    if _NC.get(has_bq) is None:
        nc = bacc.Bacc("TRN2", target_bir_lowering=False, debug=False)
        io = {}
        io["x"] = nc.dram_tensor("x", [BLOC, C, HW], F32, kind="ExternalInput").ap()
        for key in ("wu8", "wvo8"):
            io[key] = nc.dram_tensor(key, [128, CT, C], F8, kind="ExternalInput").ap()
        if has_bq:
            io["w28"] = nc.dram_tensor("w28", [128, CT, 16], F8, kind="ExternalInput").ap()
        io["gmask"] = nc.dram_tensor("gmask", [C, G], F32R, kind="ExternalInput").ap()
        io["gmaskT"] = nc.dram_tensor("gmaskT", [G, C], F32R, kind="ExternalInput").ap()
        io["ones8"] = nc.dram_tensor("ones8", [128, 2, 16], F8, kind="ExternalInput").ap()
        io["ones1"] = nc.dram_tensor("ones1", [1, 128], F32R, kind="ExternalInput").ap()
        io["vecs"] = nc.dram_tensor("vecs", [C, 4], F32, kind="ExternalInput").ap()
        io["out"] = nc.dram_tensor("out", [BLOC, C, HW], F32, kind="ExternalOutput").ap()
        with tile.TileContext(nc, pool_alloc_mode="queue") as tc:
            _emit(tc, io)
        nc.compile()
        _NC[has_bq] = nc
    return _NC[has_bq]


def _pack8(w):
    # [C, F] -> [128, CT, F] fp8 (partition p, plane t) <- row t*128+p
    w = np.clip(np.asarray(w, np.float64), -F8MAX, F8MAX).astype(np.float32)
    return np.ascontiguousarray(
        w.reshape(CT, 128, -1).transpose(1, 0, 2)
    ).astype(NF8)


def _host_prep(x, gn_w, gn_b, wq, bq, wk, bk, wv, bv, wo, bo):
    f = np.float32
    wq64 = np.asarray(wq, np.float64)
    wk64 = np.asarray(wk, np.float64)
    wv64 = np.asarray(wv, np.float64)
    wo64 = np.asarray(wo, np.float64)
    has_bq = bool(np.any(np.asarray(bq) != 0))
    shared = {
        "wu8": _pack8(SU * SCALE * (wq64.T @ wk64)),
        "wvo8": _pack8(SV * (wo64 @ wv64).T),
        "vecs": np.ascontiguousarray(
            np.stack(
                [
                    np.asarray(bq, dtype=f),
                    np.asarray(gn_w, dtype=f),
                    np.asarray(gn_b, dtype=f),
                    (bo + wo @ bv).astype(f),
                ],
                axis=1,
            )
        ),
        "gmask": np.repeat(np.eye(G, dtype=f), GSZ, axis=0),
        "gmaskT": np.ascontiguousarray(np.repeat(np.eye(G, dtype=f), GSZ, axis=0).T),
        "ones8": np.ones((128, 2, 16), dtype=NF8),
        "ones1": np.full((1, 128), SV, dtype=f),
    }
    if has_bq:
        shared["w28"] = _pack8(
            np.repeat(
                (SU2 * SCALE * (wk64.T @ np.asarray(bq, np.float64)))[:, None], 16, 1
            )
        )
    xr = np.ascontiguousarray(np.asarray(x, dtype=f).reshape(B, C, HW))
    in_maps = []
    for core in range(NCORES):
        m = dict(shared)
        m["x"] = np.ascontiguousarray(xr[core * BLOC : (core + 1) * BLOC])
        in_maps.append(m)
    return in_maps


def _run(inputs, trace=False, **kw):
    in_maps = _host_prep(**inputs)
    nc = _build(has_bq="w28" in in_maps[0])
    res = run_bass_kernel_spmd(
        nc, in_maps, core_ids=list(range(NCORES)), trace=trace, **kw
    )
    outs = [np.asarray(res.results[i]["out"]) for i in range(NCORES)]
    full = np.concatenate(outs, axis=0).reshape(B, C, H, W).astype(np.float32)
    return full, res


def kernel(**inputs):
    full, _ = _run(inputs, trace=False)
    return full


# revision 12
# speedup vs baseline: 1.3449x; 1.0458x over previous
"""AttentionBlock (GroupNorm + single-head self-attention + residual) on 8 trn2 cores.

Data-parallel over batch: B=16 images, 2 per core; no collectives. All large
matmuls run as fp8-e4m3 with perf_mode=DoubleRow: operands are packed
[128, planes, free] and each matmul contracts 2 planes (K=256) at once,
~1.4-2x the fp32r/bf16 PE rate. Host-side scaling keeps every fp8 operand at
unit-ish sigma (wuT x512, wvoT x16), compensated exactly on-device by the exp
scale (1/512) and by folding 1/16 into the softmax-denominator reciprocal.
Measured end-to-end rel err ~5e-3 vs the fp32 reference (tolerance 2e-2).

The four 512x512 projections are algebraically merged HOST-SIDE into two:
  logits  = scale * q^T k = hn^T (scale * wq^T wk) hn   -> one u-projection
  output  = wo @ (attn @ v) = attn @ ((wo wv) @ hn)     -> one v'-projection
bk is dropped (constant per softmax row); bv folds into bo' = bo + wo@bv; a
nonzero bq becomes a per-m exp() bias from tiny matmuls of hn against
scale * wk^T bq (exact; bq==0 in practice skips it).

Per-image fp8 layouts (SBUF, partition x planes x free):
  hn, u  : [128, 4, 1024]  (channel planes x pixels)
  v'T    : [128, 8, 512]   (pixel planes x channels)
  attnT  : [128, 8, 1024]  (exp(L^T), pixel planes x pixels)
Softmax runs without max-subtraction (logits ~N(0,1) by construction); exp
uses bias -1.5 to keep fp8 attnT well inside e4m3 range. Denominators are
column sums of the *stored* fp8 attnT via ones-vector DoubleRow matmuls
(exact normalization consistency), broadcast to 128 partitions by a K=1
outer-product matmul + fast reciprocal.

GroupNorm runs in fp32: bn_stats/bn_aggr (one DVE pass per x tile) give
per-channel mean/var; group reduction and per-channel broadcast go through
tiny group-membership matmuls. x loads split across the two HWDGE queues
(sync/scalar), weights ride SWDGE (gpsimd/vector); a burst of junk matmuls
warms the PE clock (HAM) during the initial DMA wait. Image 1's hn is
produced mid-way through image 0's attention so the PE never idles between
images; the epilogue (1/denominator, +bias, +residual) streams per
[128, 512] chunk on DVE/GPSIMD straight into output DMAs on 4 queues.
"""

import sys

sys.path.insert(0, "/opt/trn_rl_repo")

from contextlib import ExitStack

import numpy as np
import ml_dtypes

import concourse.bass as bass
import concourse.bacc as bacc
import concourse.mybir as mybir
import concourse.tile as tile
from concourse.bass_utils import run_bass_kernel_spmd

B, C, H, W = 16, 512, 32, 32
HW = H * W  # 1024 pixels (n/m index)
NCORES = 8
BLOC = B // NCORES  # 2 images per core
G = 8  # groupnorm groups
GSZ = C // G  # 64 channels per group
SCALE = float(C) ** -0.5
EPS = 1e-5
INVCNT = 1.0 / (GSZ * HW)

SU = 512.0  # host scale on wuT; exp() divides it back out
SV = 16.0  # host scale on wvoT; folded into denominator reciprocal
SU2 = 64.0  # host scale on the bq logit-offset column
EXPB = -1.5  # exp bias: keeps fp8 attnT in e4m3 range (cancels in softmax)
F8MAX = 240.0  # TRN e4m3 max normal

F32 = mybir.dt.float32
F32R = mybir.dt.float32r
F8 = mybir.dt.float8e4
NF8 = ml_dtypes.float8_e4m3
AF = mybir.ActivationFunctionType
ALU = mybir.AluOpType
AX = mybir.AxisListType
DR = mybir.MatmulPerfMode.DoubleRow

CT = C // 128  # 4 channel tiles
CP = CT // 2  # 2 channel-tile pairs (DoubleRow K=256)
NB = HW // 128  # 8 row blocks of the attention matrix
NP = NB // 2  # 4 row-block pairs
NCH = HW // 512  # 2 free-dim chunks of 512

NWARM = 13  # junk PE warmup matmuls covering the x-DMA + stats window


def r(ap):
    return ap.bitcast(F32R)


def _emit(tc, io):
    nc = tc.nc
    with ExitStack() as ctx, nc.allow_low_precision(reason="fp8 attention"):
        wp = ctx.enter_context(tc.tile_pool(name="wp", bufs=1))
        sb = ctx.enter_context(tc.tile_pool(name="sb", bufs=1))
        sp = ctx.enter_context(tc.tile_pool(name="sp", bufs=2))
        ps_l = ctx.enter_context(tc.tile_pool(name="ps_l", bufs=2, space="PSUM"))
        ps_m = ctx.enter_context(tc.tile_pool(name="ps_m", bufs=4, space="PSUM"))

        # ---- x loads first, split across both HWDGE queues (hot path) ----
        xt = [[None] * CT for _ in range(BLOC)]
        for img in range(BLOC):
            for ct in range(CT):
                t = sb.tile([128, HW], F32, name=f"x{img}_{ct}", tag=f"x{img}_{ct}")
                xt[img][ct] = t
                q = nc.sync if ct % 2 == 0 else nc.scalar
                q.dma_start(t[:], io["x"][img, ct * 128 : (ct + 1) * 128, :])

        # ---- weights / constants on the SWDGE queues ----
        wu8 = wp.tile([128, CT, C], F8, name="wu8", tag="wu8")
        nc.gpsimd.dma_start(wu8[:], io["wu8"][:])
        wvo8 = wp.tile([128, CT, C], F8, name="wvo8", tag="wvo8")
        nc.gpsimd.dma_start(wvo8[:], io["wvo8"][:])
        gmask_sb = []
        for kt in range(CT):
            t = wp.tile([128, G], F32R, name=f"gmask{kt}", tag=f"gmask{kt}")
            nc.gpsimd.dma_start(t[:], io["gmask"][kt * 128 : (kt + 1) * 128, :])
            gmask_sb.append(t)
        gmaskT_sb = wp.tile([G, C], F32R, name="gmaskT", tag="gmaskT")
        nc.gpsimd.dma_start(gmaskT_sb[:], io["gmaskT"][:])
        ones8 = wp.tile([128, 2, 16], F8, name="ones8", tag="ones8")
        nc.gpsimd.dma_start(ones8[:], io["ones8"][:])
        ones1 = wp.tile([1, 128], F32R, name="ones1", tag="ones1")
        nc.gpsimd.dma_start(ones1[:], io["ones1"][:])
        vecs_sb = wp.tile([128, CT * 4], F32, name="vecs", tag="vecs")
        nc.gpsimd.dma_start(
            vecs_sb[:].rearrange("p (t f) -> p t f", t=CT),
            io["vecs"].rearrange("(t p) f -> p t f", p=128),
        )
        w2_sb = None
        if io.get("w28") is not None:
            w2_sb = wp.tile([128, CT, 16], F8, name="w28", tag="w28")
            nc.gpsimd.dma_start(w2_sb[:], io["w28"][:])

        def vcol(ct, f):
            return vecs_sb[:, ct * 4 + f : ct * 4 + f + 1]

        # PE warmup: the array sits idle ~5us waiting on x-DMA + GN stats and
        # would start cold (HAM 1.2GHz). Fill the window with junk fp32r
        # matmuls so the 3.4us activity window is warm when real work arrives.
        wsrc = wp.tile([128, 512], F32, name="wsrc", tag="wsrc")
        nc.vector.memset(wsrc[:], 0.0)
        expb = wp.tile([128, 1], F32, name="expb", tag="expb")
        nc.vector.memset(expb[:], EXPB)
        warm_ps = ps_m.tile([128, 512], F32, name="warm_ps", tag="mm")
        for _ in range(NWARM):
            nc.tensor.matmul(
                warm_ps[:], r(wsrc[:, 0:128]), r(wsrc[:]), start=True, stop=True
            )

        # ---- groupnorm stats: per-channel sum (DVE) + sumsq (ACT / DVE) ----
        # img 0 puts sumsq on ACT (parallel with DVE sums, fastest to first
        # matmul); img 1 stays DVE-only so it never blocks ACT's u0-copy/exp
        # stream mid-kernel.
        def stats_sums(img):
            sts = []
            for ct in range(CT):
                st = sp.tile([128, 2], F32R, name="st", tag=f"st{img}_{ct}", bufs=1)
                nc.vector.reduce_sum(st[:, 0:1], xt[img][ct][:], axis=AX.X)
                scr = sp.tile([128, HW], F32, name="scr", tag=f"scr{ct % 2}", bufs=1)
                if img == 0:
                    nc.scalar.activation(
                        scr[:], xt[img][ct][:], AF.Square, accum_out=st[:, 1:2]
                    )
                else:
                    nc.vector.scalar_tensor_tensor(
                        scr[:], xt[img][ct][:], 1.0, xt[img][ct][:],
                        op0=ALU.mult, op1=ALU.mult,
                        accum_out=st[:, 1:2],
                    )
                sts.append(st)
            return sts

        def stats_phase(img, sts):
            # group sums via membership-mask matmul: [8, 2]
            gstat = ps_m.tile([G, 2], F32, name="gstat", tag="mm")
            for ct in range(CT):
                nc.tensor.matmul(
                    gstat[:], gmask_sb[ct][:], r(sts[ct][:]),
                    start=(ct == 0), stop=(ct == CT - 1),
                )
            gs = sp.tile([G, 2], F32, name="gs", tag=f"gs{img}", bufs=1)
            nc.vector.tensor_copy(gs[:], gstat[:])

            # per-group mean / rstd, packed as grp2 = [mean, rstd]
            grp2 = sp.tile([G, 2], F32R, name="grp2", tag=f"grp2{img}", bufs=1)
            tmx = sp.tile([G, 4], F32, name="tmx", tag=f"tmx{img}", bufs=1)
            nc.vector.tensor_scalar_mul(grp2[:, 0:1], gs[:, 0:1], INVCNT)  # mean
            nc.vector.tensor_scalar_mul(tmx[:, 0:1], gs[:, 1:2], INVCNT)  # E[x^2]
            nc.vector.tensor_mul(tmx[:, 1:2], grp2[:, 0:1], grp2[:, 0:1])  # mean^2
            nc.vector.scalar_tensor_tensor(
                tmx[:, 2:3], tmx[:, 0:1], EPS, tmx[:, 1:2],
                op0=ALU.add, op1=ALU.subtract,
            )  # var + eps
            nc.vector.reciprocal(tmx[:, 3:4], tmx[:, 2:3])
            nc.scalar.sqrt(grp2[:, 1:2], tmx[:, 3:4])  # rstd

            # broadcast mean/rstd to channels, fold gamma/beta
            ac, bc = [], []
            for ct in range(CT):
                bcp = ps_m.tile([128, 2], F32, name="bcp", tag="mm")
                nc.tensor.matmul(
                    bcp[:], gmaskT_sb[:, ct * 128 : (ct + 1) * 128], r(grp2[:]),
                    start=True, stop=True,
                )
                a1 = sp.tile([128, 4], F32, name=f"ab{img}{ct}", tag=f"ab{img}{ct}", bufs=1)
                # a = rstd * gamma ; b = beta - mean * a
                nc.vector.tensor_mul(a1[:, 0:1], bcp[:, 1:2], vcol(ct, 1))
                nc.vector.tensor_mul(a1[:, 2:3], bcp[:, 0:1], a1[:, 0:1])
                nc.vector.tensor_sub(a1[:, 1:2], vcol(ct, 2), a1[:, 2:3])
                ac.append(a1[:, 0:1])
                bc.append(a1[:, 1:2])
            return ac, bc

        # hn = x * a + b, written fp8 into plane ct of the packed tile
        hns = [None] * BLOC

        def emit_hn(img, ab, engines):
            ac, bc = ab
            hnp = sb.tile([128, CT, HW], F8, name=f"hn{img}", tag=f"hn{img}")
            for ct in range(CT):
                dst = hnp[:, ct, :]
                eng = engines[ct % len(engines)]
                if eng is nc.scalar:
                    nc.scalar.activation(
                        dst, xt[img][ct][:], AF.Identity, bias=bc[ct], scale=ac[ct]
                    )
                else:
                    eng.tensor_scalar(
                        dst, xt[img][ct][:], ac[ct], bc[ct], op0=ALU.mult, op1=ALU.add
                    )
            hns[img] = hnp

        def pcopy(i, dst, src):
            # PSUM -> SBUF copy (with fp8 convert); GPSIMD can't read PSUM
            if i % 2 == 0:
                nc.vector.tensor_copy(dst, src)
            else:
                nc.scalar.copy(dst, src)

        def emit_tv(img):
            # optional per-m logit offset for nonzero bq
            if w2_sb is None:
                return None
            hnp = hns[img]
            tv_sb = []
            for mt in range(NB):
                tvp = ps_m.tile([128, 1], F32, name="tvp", tag="mm")
                for t in range(CP):
                    nc.tensor.matmul(
                        tvp[:],
                        hnp[:, 2 * t : 2 * t + 2, mt * 128 : (mt + 1) * 128],
                        w2_sb[:, 2 * t : 2 * t + 2, 0:1],
                        start=(t == 0), stop=(t == CP - 1), perf_mode=DR,
                    )
                tb = sp.tile([128, 1], F32, name=f"tv{mt}", tag=f"tv{mt}", bufs=2)
                nc.vector.tensor_scalar(
                    tb[:], tvp[:], 1.0 / SU2, EXPB, op0=ALU.mult, op1=ALU.add
                )
                tv_sb.append(tb)
            return tv_sb

        ups = [None] * BLOC
        vTps = [None] * BLOC
        attnps = [None] * BLOC

        def emit_u(img):
            # u projection: u = (SU * scale * wk^T wq) @ hn
            hnp = hns[img]
            up = sb.tile([128, CT, HW], F8, name=f"u{img}", tag=f"u{img}")
            ups[img] = up
            ci = 0
            for cc in range(CT):
                accs = [
                    ps_m.tile([128, 512], F32, name="qp", tag="mm") for _ in range(NCH)
                ]
                for t in range(CP):
                    for nch in range(NCH):
                        nc.tensor.matmul(
                            accs[nch][:],
                            wu8[:, 2 * t : 2 * t + 2, cc * 128 : (cc + 1) * 128],
                            hnp[:, 2 * t : 2 * t + 2, nch * 512 : (nch + 1) * 512],
                            start=(t == 0), stop=(t == CP - 1), perf_mode=DR,
                        )
                for nch in range(NCH):
                    pcopy(ci, up[:, cc, nch * 512 : (nch + 1) * 512], accs[nch][:])
                    ci += 1

        def emit_vT(img, mts):
            # v'T: [m, c'] with v' = (SV * wo wv) @ hn
            hnp = hns[img]
            if vTps[img] is None:
                vTps[img] = sb.tile([128, NB, C], F8, name=f"vT{img}", tag=f"vT{img}")
            vTp = vTps[img]
            for i, mt in enumerate(mts):
                acc = ps_m.tile([128, 512], F32, name="vp", tag="mm")
                for t in range(CP):
                    nc.tensor.matmul(
                        acc[:],
                        hnp[:, 2 * t : 2 * t + 2, mt * 128 : (mt + 1) * 128],
                        wvo8[:, 2 * t : 2 * t + 2, :],
                        start=(t == 0), stop=(t == CP - 1), perf_mode=DR,
                    )
                pcopy(mt, vTp[:, mt, :], acc[:])

        def emit_logits(img, tv_sb):
            # attention: L^T = hn^T u per m-tile; exp writes fp8 attnT
            hnp = hns[img]
            up = ups[img]
            attnp = sb.tile([128, NB, HW], F8, name=f"at{img}", tag=f"at{img}")
            attnps[img] = attnp
            for mt in range(NB):
                lpT = ps_l.tile([128, HW], F32, name="lpT", tag="lpT")
                for t in range(CP):
                    for nch in range(NCH):
                        nc.tensor.matmul(
                            lpT[:, nch * 512 : (nch + 1) * 512],
                            hnp[:, 2 * t : 2 * t + 2, mt * 128 : (mt + 1) * 128],
                            up[:, 2 * t : 2 * t + 2, nch * 512 : (nch + 1) * 512],
                            start=(t == 0), stop=(t == CP - 1), perf_mode=DR,
                        )
                bias = tv_sb[mt][:] if tv_sb is not None else expb[:]
                nc.scalar.activation(
                    attnp[:, mt, :], lpT[:], AF.Exp, bias=bias, scale=1.0 / SU
                )

        def emit_cs_rb(img):
            # softmax denominators: column sums of stored fp8 attnT, then
            # 1/(SV*sum) broadcast to all partitions via outer-product matmul
            attnp = attnps[img]
            cs_t = []
            for half in range(2):
                hsl = slice(half * 512, (half + 1) * 512)
                cs = ps_m.tile([1, 512], F32, name="cs", tag="mm")
                for j in range(NP):
                    nc.tensor.matmul(
                        cs[:],
                        ones8[:, :, 0:1],
                        attnp[:, 2 * j : 2 * j + 2, hsl],
                        start=(j == 0), stop=(j == NP - 1), perf_mode=DR,
                    )
                cs_t.append(cs)
            rrows = []
            for half in range(2):
                rrow = sp.tile([1, 512], F32R, name="rrow", tag=f"rrow{half}", bufs=2)
                nc.vector.tensor_copy(rrow[:], cs_t[half][:])
                rrows.append(rrow)
            rb_sb = []
            for half in range(2):
                rb_ps = ps_m.tile([128, 512], F32, name="rb_ps", tag="mm")
                nc.tensor.matmul(
                    rb_ps[:], ones1[:], rrows[half][:], start=True, stop=True
                )
                t = sp.tile(
                    [128, 512], F32, name=f"rb{img}_{half}", tag=f"rb{img}_{half}", bufs=1
                )
                nc.vector.reciprocal_approx_fast(t[:], rb_ps[:])
                rb_sb.append(t)
            return rb_sb

        def emit_av(img, rb_sb, ccs):
            # A @ V': epilogue normalizes, adds bias + residual, streams out
            vTp = vTps[img]
            attnp = attnps[img]
            outqs = (nc.sync, nc.scalar, nc.gpsimd, nc.sync)
            for cc in ccs:
                accs = [
                    ps_m.tile([128, 512], F32, name="op", tag="mm") for _ in range(2)
                ]
                for j in range(NP):
                    for half in range(2):
                        nc.tensor.matmul(
                            accs[half][:],
                            vTp[:, 2 * j : 2 * j + 2, cc * 128 : (cc + 1) * 128],
                            attnp[:, 2 * j : 2 * j + 2, half * 512 : (half + 1) * 512],
                            start=(j == 0), stop=(j == NP - 1), perf_mode=DR,
                        )
                for half in range(2):
                    hsl = slice(half * 512, (half + 1) * 512)
                    on = sp.tile([128, 512], F32, name="on", tag="on", bufs=3)
                    nc.vector.tensor_mul(on[:], accs[half][:], rb_sb[half][:])
                    res = sp.tile([128, 512], F32, name="res", tag="res", bufs=3)
                    nc.vector.scalar_tensor_tensor(
                        res[:], on[:], vcol(cc, 3), xt[img][cc][:, hsl],
                        op0=ALU.add, op1=ALU.add,
                    )
                    outqs[(cc * 2 + half) % 4].dma_start(
                        io["out"][img, cc * 128 : (cc + 1) * 128, hsl], res[:]
                    )

        # ---------- schedule ----------
        # Emission order == per-engine execution order. The plan keeps the PE
        # stream dense (cross-image interleave fills every wait window) and
        # keeps critical DVE/ACT producers ahead of bulk work in their queues.
        sts0 = stats_sums(0)                  # DVE sums + ACT squares
        ab0 = stats_phase(0, sts0)            # PE gstat0/bcast0 + DVE chain
        emit_hn(0, ab0, (nc.vector, nc.scalar))
        tv0 = emit_tv(0)
        emit_u(0)
        emit_vT(0, range(0, NB // 2))
        emit_logits(0, tv0)                   # ACT: exps0 chase the PE
        sts1 = stats_sums(1)                  # DVE-only; runs during u0/vT0a
        ab1 = stats_phase(1, sts1)            # PE tiny; lands after L0
        emit_hn(1, ab1, (nc.vector,))
        emit_vT(0, range(NB // 2, NB))        # PE filler while exps0 drain
        tv1 = emit_tv(1)
        emit_u(1)                             # more filler; hn1 just built
        rb0 = emit_cs_rb(0)                   # exps0 done by now -> no stall
        emit_av(0, rb0, (0, 1))
        emit_vT(1, range(0, NB // 2))
        emit_logits(1, tv1)                   # ACT: exps1
        emit_vT(1, range(NB // 2, NB))
        emit_av(0, rb0, (2, 3))               # PE filler while exps1 drain
        rb1 = emit_cs_rb(1)
        emit_av(1, rb1, (0, 1, 2, 3))



_NC = {}


def _build(has_bq=False):
    global _NC
    if _NC.get(has_bq) is None:
        nc = bacc.Bacc("TRN2", target_bir_lowering=False, debug=False)
        io = {}
        io["x"] = nc.dram_tensor("x", [BLOC, C, HW], F32, kind="ExternalInput").ap()
        for key in ("wu8", "wvo8"):
            io[key] = nc.dram_tensor(key, [128, CT, C], F8, kind="ExternalInput").ap()
        if has_bq:
            io["w28"] = nc.dram_tensor("w28", [128, CT, 16], F8, kind="ExternalInput").ap()
        io["gmask"] = nc.dram_tensor("gmask", [C, G], F32R, kind="ExternalInput").ap()
        io["gmaskT"] = nc.dram_tensor("gmaskT", [G, C], F32R, kind="ExternalInput").ap()
        io["ones8"] = nc.dram_tensor("ones8", [128, 2, 16], F8, kind="ExternalInput").ap()
        io["ones1"] = nc.dram_tensor("ones1", [1, 128], F32R, kind="ExternalInput").ap()
        io["vecs"] = nc.dram_tensor("vecs", [C, 4], F32, kind="ExternalInput").ap()
        io["out"] = nc.dram_tensor("out", [BLOC, C, HW], F32, kind="ExternalOutput").ap()
        with tile.TileContext(nc, pool_alloc_mode="queue") as tc:
            _emit(tc, io)
        nc.compile()
        _NC[has_bq] = nc
    return _NC[has_bq]


def _pack8(w):
    # [C, F] -> [128, CT, F] fp8 (partition p, plane t) <- row t*128+p
    w = np.clip(np.asarray(w, np.float64), -F8MAX, F8MAX).astype(np.float32)
    return np.ascontiguousarray(
        w.reshape(CT, 128, -1).transpose(1, 0, 2)
    ).astype(NF8)


def _host_prep(x, gn_w, gn_b, wq, bq, wk, bk, wv, bv, wo, bo):
    f = np.float32
    wq64 = np.asarray(wq, np.float64)
    wk64 = np.asarray(wk, np.float64)
    wv64 = np.asarray(wv, np.float64)
    wo64 = np.asarray(wo, np.float64)
    has_bq = bool(np.any(np.asarray(bq) != 0))
    shared = {
        "wu8": _pack8(SU * SCALE * (wq64.T @ wk64)),
        "wvo8": _pack8(SV * (wo64 @ wv64).T),
        "vecs": np.ascontiguousarray(
            np.stack(
                [
                    np.asarray(bq, dtype=f),
                    np.asarray(gn_w, dtype=f),
                    np.asarray(gn_b, dtype=f),
                    (bo + wo @ bv).astype(f),
                ],
                axis=1,
            )
        ),
        "gmask": np.repeat(np.eye(G, dtype=f), GSZ, axis=0),
        "gmaskT": np.ascontiguousarray(np.repeat(np.eye(G, dtype=f), GSZ, axis=0).T),
        "ones8": np.ones((128, 2, 16), dtype=NF8),
        "ones1": np.full((1, 128), SV, dtype=f),
    }
    if has_bq:
        shared["w28"] = _pack8(
            np.repeat(
                (SU2 * SCALE * (wk64.T @ np.asarray(bq, np.float64)))[:, None], 16, 1
            )
        )
    xr = np.ascontiguousarray(np.asarray(x, dtype=f).reshape(B, C, HW))
    in_maps = []
    for core in range(NCORES):
        m = dict(shared)
        m["x"] = np.ascontiguousarray(xr[core * BLOC : (core + 1) * BLOC])
        in_maps.append(m)
    return in_maps


def _run(inputs, trace=False, **kw):
    in_maps = _host_prep(**inputs)
    nc = _build(has_bq="w28" in in_maps[0])
    res = run_bass_kernel_spmd(
        nc, in_maps, core_ids=list(range(NCORES)), trace=trace, **kw
    )
    outs = [np.asarray(res.results[i]["out"]) for i in range(NCORES)]
    full = np.concatenate(outs, axis=0).reshape(B, C, H, W).astype(np.float32)
    return full, res


def kernel(**inputs):
    full, _ = _run(inputs, trace=False)
    return full
